# revision 14
# baseline (speedup 1.0000x reference)
"""Causal single-head attention on 8 trn2 NeuronCores, data-parallel over batch.

Reference computation (per batch element b):
  Q = x_b @ Wq.T + bq ; K = x_b @ Wk.T + bk ; V = x_b @ Wv.T + bv    (S=2048, D=A=1024)
  out_b = softmax(causal(Q K^T / 32)) V

Sharding: x is (S, B=8, D); core c handles batch element c. No collectives.

Per-core kernel design (v2, bf16 datapath):
  - host passes xT = x[:,c,:].T (D,S) and pre-transposed weights WT (D,A) in bf16
  - all of xT, KT (A,S), QT (A,S), V (S,A) are SBUF-resident in bf16 (no spills,
    x and W are read from HBM exactly once)
  - KT/QT[a,s] = WT_chunk.T @ xT chunks; V[s,a] = xT_chunk.T @ WvT chunks; psum
    is fp32, evacuation on ScalarE fuses the per-partition bias add (Identity)
  - attention in scores-transposed orientation: ST[k, q] = KT_chunk.T @ QT_block.
    exp(ST) on ScalarE (no row-max subtraction: scores have std ~0.2, |s| < ~2,
    exp is safe) is directly the stationary operand for out[q,a] += P_chunk.T @
    V_chunk - no P transposes anywhere
  - softmax denominator: VectorE accumulates the P chunks into den_acc[128, QB]
    (fp32), then one ones-rhs matmul pair per q-block reduces the partition dim;
    normalization is folded into the PSUM->SBUF output evacuation scale (ScalarE)
  - every matmul accumulation chain owns a whole PSUM bank and runs
    uninterleaved (interleaved chains measurably stall the PE on this HW)
  - weight pool is double-buffered so the next projection's weights stream in
    behind the current projection's matmuls
"""
import numpy as np

S = 2048
D = 1024
A = 1024
B = 8
QB = 256          # attention query-block width
NQB = S // QB     # 8
KC = 128          # attention key-chunk width
SCALE = 1.0 / 32.0  # 1/sqrt(A)
NEG = -1e30

_cache = {}


def _emit_body(nc, tc):
    import concourse.bass as bass
    import concourse.mybir as mybir

    f32 = mybir.dt.float32
    bf16 = mybir.dt.bfloat16
    AF = mybir.ActivationFunctionType

    xTd = nc.tensors["xT"]
    wqT, wkT, wvT = nc.tensors["wqT"], nc.tensors["wkT"], nc.tensors["wvT"]
    bq, bk, bv = nc.tensors["bq"], nc.tensors["bk"], nc.tensors["bv"]
    mask, ones2, out = nc.tensors["mask"], nc.tensors["ones2"], nc.tensors["out"]

    def bcast_ap(handle, n_part, n_free):
        ap = handle[:]
        return bass.AP(tensor=ap.tensor, offset=ap.offset, ap=[[0, n_part], [1, n_free]])

    with (
        tc.tile_pool(name="const", bufs=1) as cp,
        tc.tile_pool(name="kt", bufs=8) as ktp,
        tc.tile_pool(name="qt", bufs=8) as qtp,
        tc.tile_pool(name="v", bufs=16) as vp,
        tc.tile_pool(name="xt", bufs=8) as xtp,
        tc.tile_pool(name="w", bufs=2) as wp,
    ):
        kt = [ktp.tile([128, S], bf16, tag="kt", name=f"kt{i}") for i in range(8)]
        qt = [qtp.tile([128, S], bf16, tag="qt", name=f"qt{i}") for i in range(8)]
        v = [vp.tile([128, A], bf16, tag="v", name=f"v{i}") for i in range(16)]
        xt = [xtp.tile([128, S], bf16, tag="xt", name=f"xt{i}") for i in range(8)]

        # startup: K-weights on the sync queue, first x slice on the scalar
        # queue (separate HWDGE pipelines), rest of x streams behind
        wk = wp.tile([128, 8, A], bf16, tag="w", name="wk")
        for d in range(8):
            nc.sync.dma_start(
                out=wk[:, d, :],
                in_=wkT[d * 128 : (d + 1) * 128, :],
            )
            nc.scalar.dma_start(
                out=xt[d][:, 0:512],
                in_=xTd[d * 128 : (d + 1) * 128, 0:512],
            )
        for d in range(8):
            nc.scalar.dma_start(
                out=xt[d][:, 512:2048],
                in_=xTd[d * 128 : (d + 1) * 128, 512:2048],
            )

        bq_t = cp.tile([128, 8], f32, tag="bq")
        bk_t = cp.tile([128, 8], f32, tag="bk")
        for a in range(8):
            nc.gpsimd.dma_start(
                out=bq_t[:, a : a + 1],
                in_=bq[a * 128 : (a + 1) * 128].rearrange("(p one) -> p one", one=1),
            )
            nc.gpsimd.dma_start(
                out=bk_t[:, a : a + 1],
                in_=bk[a * 128 : (a + 1) * 128].rearrange("(p one) -> p one", one=1),
            )
        bv_t = cp.tile([128, A], f32, tag="bv")
        nc.gpsimd.dma_start(out=bv_t, in_=bcast_ap(bv, 128, A))
        ones_t = cp.tile([128, 2], f32, tag="ones")
        nc.gpsimd.dma_start(out=ones_t, in_=bcast_ap(ones2, 128, 2))
        mk = [cp.tile([128, QB], f32, tag=f"mk{i}", name=f"mk{i}") for i in range(2)]
        for i in range(2):
            nc.gpsimd.dma_start(out=mk[i], in_=mask[i * 128 : (i + 1) * 128, :])

        # ---- K and Q projections: KT/QT[a,s] = sum_d WT[d,a]^T xT[d,s] + b[a] ----
        proj_psp = tc.tile_pool(name="ps", bufs=4, space="PSUM")
        psp = proj_psp.__enter__()

        def proj_qk(w, bias_t, dest):
            for s8 in range(8):
                for a in range(8):
                    ps = psp.tile([128, 256], f32, tag="ps")
                    for d in range(8):
                        nc.tensor.matmul(
                            ps,
                            w[:, d, a * 128 : (a + 1) * 128],
                            xt[d][:, s8 * 256 : (s8 + 1) * 256],
                            start=(d == 0),
                            stop=(d == 7),
                        )
                    nc.scalar.activation(
                        dest[a][:, s8 * 256 : (s8 + 1) * 256], ps, AF.Identity,
                        bias=bias_t[:, a : a + 1],
                    )

        proj_qk(wk, bk_t, kt)

        wq = wp.tile([128, 8, A], bf16, tag="w", name="wq")
        for d in range(8):
            nc.sync.dma_start(out=wq[:, d, :], in_=wqT[d * 128 : (d + 1) * 128, :])
        proj_qk(wq, bq_t, qt)

        # ---- V projection: V[s,a] = sum_d xT[d,s]^T WvT[d,a] + bv[a] ----
        wv = wp.tile([128, 8, A], bf16, tag="w", name="wv")
        for d in range(8):
            nc.sync.dma_start(out=wv[:, d, :], in_=wvT[d * 128 : (d + 1) * 128, :])
        for s_idx in range(16):
            s4, sc = divmod(s_idx, 4)
            for ah in range(2):
                ps = psp.tile([128, 512], f32, tag="ps")
                for d in range(8):
                    nc.tensor.matmul(
                        ps,
                        xt[d][:, s_idx * 128 : (s_idx + 1) * 128],
                        wv[:, d, ah * 512 : (ah + 1) * 512],
                        start=(d == 0),
                        stop=(d == 7),
                    )
                nc.vector.tensor_add(
                    v[s_idx][:, ah * 512 : (ah + 1) * 512],
                    ps,
                    bv_t[:, ah * 512 : (ah + 1) * 512],
                )

        proj_psp.__exit__(None, None, None)

        # ---- attention ----
        with (
            tc.tile_pool(name="pch", bufs=18) as pp,
            tc.tile_pool(name="stt", bufs=3) as sttp,
            tc.tile_pool(name="dac", bufs=2) as dap,
            tc.tile_pool(name="ob", bufs=6) as obp,
            tc.tile_pool(name="rin", bufs=4) as rp,
            tc.tile_pool(name="pst", bufs=2, space="PSUM") as pstp,
            tc.tile_pool(name="po", bufs=1, space="PSUM") as pop,
            tc.tile_pool(name="pd", bufs=1, space="PSUM") as pdp,
        ):
            for qb in range(NQB):
                nkc = (qb + 1) * QB // KC
                po = [
                    pop.tile([128, 512], f32, tag=f"po{i}", name=f"po{i}")
                    for i in range(4)
                ]
                den_acc = dap.tile([128, QB], f32, tag="dac")

                pchs = []
                for kc in range(nkc):
                    ps = pstp.tile([128, QB], f32, tag="st")
                    for a in range(8):
                        nc.tensor.matmul(
                            ps,
                            kt[a][:, kc * KC : (kc + 1) * KC],
                            qt[a][:, qb * QB : (qb + 1) * QB],
                            start=(a == 0),
                            stop=(a == 7),
                        )
                    pch = pp.tile([128, QB], bf16, tag="p", name=f"p{kc % 18}")
                    if kc >= 2 * qb:  # diagonal 256x256 block: apply causal mask
                        mrow = kc - 2 * qb
                        stt = sttp.tile([128, QB], f32, tag="stt")
                        nc.vector.tensor_add(stt, ps, mk[mrow])
                        nc.scalar.activation(pch, stt, AF.Exp, scale=SCALE)
                    else:
                        nc.scalar.activation(pch, ps, AF.Exp, scale=SCALE)
                    # denominator partial: den_acc += pch (fp32 accumulate on DVE)
                    if kc == 0:
                        nc.vector.tensor_copy(den_acc, pch)
                    else:
                        nc.vector.tensor_add(den_acc, den_acc, pch)
                    pchs.append(pch)

                # denominator first (den_acc is complete as soon as the last
                # exp lands): reduce over partitions via ones-rhs matmuls
                pd = [
                    pdp.tile([128, 2], f32, tag=f"pd{i}", name=f"pd{i}")
                    for i in range(2)
                ]
                rinvs = []
                for qs in range(2):
                    nc.tensor.matmul(
                        pd[qs],
                        den_acc[:, qs * 128 : (qs + 1) * 128],
                        ones_t,
                        start=True,
                        stop=True,
                    )
                    rinv = rp.tile([128, 1], f32, tag="rinv")
                    nc.vector.reciprocal(rinv, pd[qs][:, 0:1])
                    rinvs.append(rinv)

                # PV: uninterleaved accumulation chains per (q-sub, a-half);
                # evacuate each bank right after its chain so the tail is short
                osbs = [obp.tile([128, A], f32, tag="ob", name=f"ob{i}") for i in range(2)]
                for qs in range(2):
                    for ah in range(2):
                        dst = po[qs * 2 + ah]
                        for kc in range(nkc):
                            nc.tensor.matmul(
                                dst,
                                pchs[kc][:, qs * 128 : (qs + 1) * 128],
                                v[kc][:, ah * 512 : (ah + 1) * 512],
                                start=(kc == 0),
                                stop=(kc == nkc - 1),
                            )
                        nc.scalar.activation(
                            osbs[qs][:, ah * 512 : (ah + 1) * 512],
                            dst,
                            AF.Copy,
                            scale=rinvs[qs],
                        )
                    row = qb * QB + qs * 128
                    nc.gpsimd.dma_start(out=out[row : row + 128, :], in_=osbs[qs])


def _build(repeat=1):
    from concourse import bacc
    import concourse.mybir as mybir
    import concourse.tile as tile

    f32 = mybir.dt.float32
    bf16 = mybir.dt.bfloat16

    nc = bacc.Bacc("TRN2", target_bir_lowering=False)
    nc.tensors = {}
    nc.tensors["xT"] = nc.dram_tensor("xT", [D, S], bf16, kind="ExternalInput")
    nc.tensors["wqT"] = nc.dram_tensor("wqT", [D, A], bf16, kind="ExternalInput")
    nc.tensors["wkT"] = nc.dram_tensor("wkT", [D, A], bf16, kind="ExternalInput")
    nc.tensors["wvT"] = nc.dram_tensor("wvT", [D, A], bf16, kind="ExternalInput")
    nc.tensors["bq"] = nc.dram_tensor("bq", [A], f32, kind="ExternalInput")
    nc.tensors["bk"] = nc.dram_tensor("bk", [A], f32, kind="ExternalInput")
    nc.tensors["bv"] = nc.dram_tensor("bv", [A], f32, kind="ExternalInput")
    nc.tensors["mask"] = nc.dram_tensor("mask", [QB, QB], f32, kind="ExternalInput")
    nc.tensors["ones2"] = nc.dram_tensor("ones2", [2], f32, kind="ExternalInput")
    nc.tensors["out"] = nc.dram_tensor("out", [S, A], f32, kind="ExternalOutput")

    with tile.TileContext(nc) as tc:
        if repeat > 1:
            with tc.For_i(0, repeat, 1):
                _emit_body(nc, tc)
        else:
            _emit_body(nc, tc)

    nc.finalize()
    return nc


def _prep_in_maps(x, Wq, bq, Wk, bk, Wv, bv):
    """Build per-core input maps (host-side shard + layout/dtype transforms)."""
    import ml_dtypes

    bf = ml_dtypes.bfloat16
    x = np.asarray(x, dtype=np.float32)
    wqT = np.ascontiguousarray(np.asarray(Wq, dtype=np.float32).T.astype(bf))
    wkT = np.ascontiguousarray(np.asarray(Wk, dtype=np.float32).T.astype(bf))
    wvT = np.ascontiguousarray(np.asarray(Wv, dtype=np.float32).T.astype(bf))
    bq = np.asarray(bq, dtype=np.float32)
    bk = np.asarray(bk, dtype=np.float32)
    bv = np.asarray(bv, dtype=np.float32)
    kq = np.arange(QB)
    mask = np.where(kq[:, None] <= kq[None, :], 0.0, NEG).astype(np.float32)
    ones2 = np.ones(2, dtype=np.float32)
    in_maps = []
    for c in range(B):
        xTc = np.ascontiguousarray(x[:, c, :].T.astype(bf))  # (D, S) bf16
        in_maps.append(
            {
                "xT": xTc, "wqT": wqT, "wkT": wkT, "wvT": wvT,
                "bq": bq, "bk": bk, "bv": bv, "mask": mask, "ones2": ones2,
            }
        )
    return in_maps


def get_nc(repeat=1):
    key = ("nc", repeat)
    if key not in _cache:
        _cache[key] = _build(repeat)
    return _cache[key]


def kernel(x, Wq, bq, Wk, bk, Wv, bv):
    from concourse.bass_utils import run_bass_kernel_spmd

    nc = get_nc()
    in_maps = _prep_in_maps(x, Wq, bq, Wk, bk, Wv, bv)
    res = run_bass_kernel_spmd(nc, in_maps, core_ids=list(range(B)))
    outs = np.stack([res.results[c]["out"] for c in range(B)], axis=0)  # (B, S, A)
    return np.ascontiguousarray(outs.transpose(1, 0, 2))  # (S, B, A)
